# revision 29
# baseline (speedup 1.0000x reference)
"""Category-specific MLP (MoE-style routing) on 8 Trainium2 NeuronCores.

Strategy (host-routed expert/data parallel):
  - Host groups the 64 samples by cat_id into per-core work so every core
    gets exactly 8 samples (2048 tokens): token-balanced.
  - Same-cat samples are merged into multi-sample "runs" using a uniform
    per-core run-size profile (e.g. [3,2,1,1,1]) chosen adaptively from the
    cat histogram, so one SPMD program serves all 8 cores. A run loads its
    4-layer weight bank once; fewer runs = less HBM traffic.
  - Weight banks are fetched as ONE whole-bank DMA per (run, layer)
    (2 MB, 2 KB/partition chunks) on the SP queue; x and bias stream on the
    DVE queue; outputs drain on the GpSimd (SWDGE) queue. The Activation
    queue carries no DMA so silu epilogues never queue behind DMA issue.
  - Activations live transposed [D, tok]; each layer computes
    out_T = W_l.T @ h_T via matmul(lhsT=W tile, rhs=h_T tile) in 512-token
    chunks (f32 PSUM, all 8 banks), silu+bias on ACT (layers 1-3), bias-add
    on DVE (layer 4). On-chip dtypes bf16; output returns bf16 and is
    upcast on host.
"""

import numpy as np
from contextlib import ExitStack

import ml_dtypes

import concourse.bass as bass
import concourse.mybir as mybir
import concourse.tile as tile
from concourse import bacc
from concourse.bass_utils import run_bass_kernel_spmd

P = 128          # SBUF partitions
D = 1024         # model dim (in = hidden = out)
KT = D // P      # 8 k-tiles per dim
TOK = 256        # tokens per sample
S = 8            # samples per core
L = 4            # layers
NCORES = 8
CMAX = 512       # matmul chunk (PSUM bank = 512 f32)

ACT_DT = mybir.dt.bfloat16
ACT_NP = ml_dtypes.bfloat16

LAST_RESULT = None
_PROGRAM_CACHE = {}


def _partitions(n, maxp=None):
    """All integer partitions of n, parts descending."""
    maxp = n if maxp is None else maxp
    if n == 0:
        yield ()
        return
    for first in range(min(n, maxp), 0, -1):
        for rest in _partitions(n - first, first):
            yield (first,) + rest


def plan(cat_ids):
    """Pick per-core sample order and the uniform run-size profile.

    Returns (order, profile): order is a [64] permutation; core c owns
    order[8c:8c+8]. profile is a descending tuple of run sizes summing to
    S; every core's samples group into runs of those sizes, each run one
    category.
    """
    cat_ids = np.asarray(cat_ids).astype(np.int64)
    by_cat = {}
    for i, c in enumerate(cat_ids.tolist()):
        by_cat.setdefault(c, []).append(i)

    def try_profile(prof):
        remaining = {c: list(v) for c, v in by_cat.items()}
        runs_by_size = []
        for s in prof:
            runs = []
            for _ in range(NCORES):
                cat = max(remaining, key=lambda c: len(remaining[c]))
                if len(remaining[cat]) < s:
                    return None
                runs.append([remaining[cat].pop() for _ in range(s)])
                if not remaining[cat]:
                    del remaining[cat]
            runs_by_size.append(runs)
        return runs_by_size

    # fewer runs first (fewer bank loads), then fewer odd parts (more
    # full-512 chunks)
    cands = sorted(_partitions(S),
                   key=lambda pr: (len(pr), sum(s % 2 for s in pr)))
    for prof in cands:
        runs_by_size = try_profile(prof)
        if runs_by_size is not None:
            break
    order = []
    for c in range(NCORES):
        for runs in runs_by_size:
            order.extend(runs[c])
    return np.asarray(order), prof


def _chunks(tok, cmax=CMAX):
    out = []
    off = 0
    while off < tok:
        c = min(cmax, tok - off)
        out.append((off, c))
        off += c
    return out


def build_program(profile=(1,) * S, reps=1, mode="full", cmax=CMAX,
                  w_bufs=None, w_split=4, xo_split=1):
    # w_split=4: each 2MB bank fetched as 4 x 512KB DMAs — duty-cycles the
    # 16-engine SDMA burst (less SBUF-port contention with PE streaming) and
    # lets matmuls start on early k-slices (range-based deps); measured
    # ~40-55us faster than whole-bank on HW.
    """One SPMD program for all 8 cores: R runs x 4 layers.

    reps>1 wraps the computation in a hardware loop (timing only).
    mode: "full" (graded); "compute_only" (weights loaded once, no
    steady-state weight DMA) / "dma_only" (no PE/epilogue work) for
    bottleneck attribution in the test harness."""
    toks = [s * TOK for s in profile]
    R = len(toks)
    offs = np.concatenate([[0], np.cumsum(toks)])
    tokmax = max(toks)

    nc = bacc.Bacc("TRN2", target_bir_lowering=False, debug=False,
                   num_devices=NCORES)
    xT = nc.dram_tensor("xT", [D, S * TOK], ACT_DT, kind="ExternalInput")
    wg = nc.dram_tensor("wg", [R, L, D, D], ACT_DT, kind="ExternalInput")
    bg = nc.dram_tensor("bg", [P, L * R * KT], mybir.dt.float32,
                        kind="ExternalInput")
    outT = nc.dram_tensor("outT", [D, S * TOK], ACT_DT,
                          kind="ExternalOutput")

    xv = xT.ap().rearrange("(k p) n -> p k n", p=P)
    ov = outT.ap().rearrange("(k p) n -> p k n", p=P)

    silu = mybir.ActivationFunctionType.Silu

    # SBUF budget guard (bytes/partition): shrink buffering for huge runs
    w_banks, h_gens, o_bufs = (w_bufs or 4), 3, 2
    def sbuf_need():
        return (w_banks * KT * D * 2 + h_gens * KT * tokmax * 2
                + (1 + o_bufs) * KT * tokmax * 2 + L * R * KT * 4)
    while sbuf_need() > 200 * 1024 and w_banks > 2:
        w_banks -= 1
    while sbuf_need() > 200 * 1024 and h_gens > 2:
        h_gens -= 1
    while sbuf_need() > 200 * 1024 and o_bufs > 1:
        o_bufs -= 1

    with tile.TileContext(nc) as tc, ExitStack() as ctx:
        wpool = ctx.enter_context(tc.tile_pool(name="w", bufs=w_banks))
        hpool = ctx.enter_context(tc.tile_pool(name="h", bufs=h_gens * KT))
        # bufs=1: run r+1's x DMA waits for run r's layer-0 reads — natural
        # just-in-time prefetch that keeps startup DMA bandwidth for run 0
        # (mm_only reads x in every layer, so it needs 2 slots)
        xpool = ctx.enter_context(
            tc.tile_pool(name="x", bufs=2 if mode == "mm_only" else 1))
        opool = ctx.enter_context(tc.tile_pool(name="o", bufs=o_bufs))
        ppool = ctx.enter_context(tc.tile_pool(name="ps", bufs=8,
                                               space="PSUM"))
        cpool = ctx.enter_context(tc.tile_pool(name="c", bufs=1))

        btile = cpool.tile([P, L * R * KT], mybir.dt.float32)

        ms = list(reversed(range(KT)))
        ks = list(range(1, KT)) + [0]

        def body(_iv=None):
            xt = {}

            def issue_x(r):
                tok, off = toks[r], int(offs[r])
                xt[r] = xpool.tile([P, KT * tokmax], ACT_DT, tag="x",
                                   name="x")
                xsrc = xv[:, :, off:off + tok]
                xdst = xt[r][:, :KT * tok].rearrange("p (k n) -> p k n", k=KT)
                if r == 0:
                    # startup: per-k tiles on the Act ring (idle pre-epilogue)
                    # so the first matmuls stream behind them
                    for k in range(KT):
                        nc.scalar.dma_start(xdst[:, k, :], xsrc[:, k, :])
                    nc.scalar.dma_start(btile[:], bg.ap()[:, :])
                else:
                    kk = KT // xo_split
                    for si in range(xo_split):
                        nc.gpsimd.dma_start(xdst[:, si * kk:(si + 1) * kk, :],
                                            xsrc[:, si * kk:(si + 1) * kk, :])

            issue_x(0)
            once_w = {}
            for r in range(R):
                tok, off = toks[r], int(offs[r])
                chs = _chunks(tok, cmax)
                hin = None      # layer 0 reads from xt[r]
                for l in range(L):
                    if mode in ("compute_only", "mm_only") and l in once_w:
                        w = once_w[l]
                    else:
                        w = wpool.tile([P, KT * D], ACT_DT, tag="w",
                                       name=f"w{r}_{l}")
                        wsrc = wg.ap()[r, l].rearrange("(k p) m -> p k m", p=P)
                        wdst = w[:, :].rearrange("p (k m) -> p k m", k=KT)
                        if r == 0 and l == 0:
                            for k in range(KT):
                                nc.sync.dma_start(wdst[:, k, :], wsrc[:, k, :])
                        elif w_split > 1:
                            kk = KT // w_split
                            for si in range(w_split):
                                nc.sync.dma_start(
                                    wdst[:, si * kk:(si + 1) * kk, :],
                                    wsrc[:, si * kk:(si + 1) * kk, :])
                        else:
                            nc.sync.dma_start(wdst[:, :, :], wsrc[:, :, :])
                        if mode in ("compute_only", "mm_only"):
                            once_w[l] = w
                    if l == 0 and r + 1 < R:
                        issue_x(r + 1)
                    last = l == L - 1
                    if mode == "mm_only":
                        pass
                    elif last:
                        otile = opool.tile([P, KT * tokmax], ACT_DT, tag="o",
                                           name=f"o{r % o_bufs}")
                    else:
                        hout = [hpool.tile([P, tokmax], ACT_DT, tag="acts",
                                           name=f"h{l}_{k}")
                                for k in range(KT)]

                    def rhs(k, c0, ctok):
                        if hin is None or mode == "mm_only":
                            return xt[r][:, k * tok + c0:k * tok + c0 + ctok]
                        return hin[k][:, c0:c0 + ctok]

                    def epilogue(m, ps, c0, ctok):
                        if mode == "mm_only":
                            return
                        col = (l * R + r) * KT + m
                        if last:
                            nc.vector.tensor_scalar_add(
                                otile[:, m * tok + c0:m * tok + c0 + ctok],
                                ps[:, :ctok], btile[:, col:col + 1])
                        else:
                            nc.scalar.activation(
                                hout[m][:, c0:c0 + ctok], ps[:, :ctok], silu,
                                bias=btile[:, col:col + 1])

                    if mode == "dma_only":
                        hin = hout if not last else None
                        continue
                    for ci, (c0, ctok) in enumerate(chs):
                        if r == 0 and l == 0 and ci == 0:
                            # k-outer: stream behind the first per-k DMAs
                            pss = [ppool.tile([P, CMAX], mybir.dt.float32,
                                              tag="ps", name=f"ps{m}")
                                   for m in range(KT)]
                            for j in range(KT):
                                for m in ms:
                                    nc.tensor.matmul(
                                        pss[m][:, :ctok],
                                        w[:, j * D + m * P:j * D + (m + 1) * P],
                                        rhs(j, c0, ctok),
                                        start=(j == 0), stop=(j == KT - 1))
                            for m in ms:
                                epilogue(m, pss[m], c0, ctok)
                        else:
                            for m in ms:
                                ps = ppool.tile([P, CMAX], mybir.dt.float32)
                                for j, k in enumerate(ks):
                                    nc.tensor.matmul(
                                        ps[:, :ctok],
                                        w[:, k * D + m * P:k * D + (m + 1) * P],
                                        rhs(k, c0, ctok),
                                        start=(j == 0), stop=(j == KT - 1))
                                epilogue(m, ps, c0, ctok)
                    if not last and mode != "mm_only":
                        hin = hout
                if mode in ("dma_only", "mm_only"):
                    continue    # otile unwritten; skip out DMA
                if r == R - 1:
                    # final run: drain per-m on the (now idle) Act ring so the
                    # tail is one small DMA past the last bias-add
                    for m in ms:
                        nc.scalar.dma_start(ov[:, m, off:off + tok],
                                            otile[:, m * tok:m * tok + tok])
                else:
                    odst = ov[:, :, off:off + tok]
                    osrc = otile[:, :KT * tok].rearrange("p (k n) -> p k n",
                                                         k=KT)
                    kk = KT // xo_split
                    for si in range(xo_split):
                        nc.gpsimd.dma_start(odst[:, si * kk:(si + 1) * kk, :],
                                            osrc[:, si * kk:(si + 1) * kk, :])

        if reps == 1:
            body()
        else:
            with tc.For_i(0, reps, 1) as iv:
                body(iv)
    nc.compile()
    return nc


def prepare_in_maps(x, cat_ids, Ws, bs, order, profile):
    x = np.asarray(x)
    cat_ids = np.asarray(cat_ids).astype(np.int64)
    toks = [s * TOK for s in profile]
    R = len(toks)
    run_first = np.concatenate([[0], np.cumsum(toks)])[:-1] // TOK
    in_maps = []
    for c in range(NCORES):
        samp = order[c * S:(c + 1) * S]
        xs = np.asarray(x[samp], dtype=np.float32)            # [S, TOK, D]
        xTc = np.ascontiguousarray(xs.reshape(S * TOK, D).T)  # [D, S*TOK]
        cats = [int(cat_ids[samp[i]]) for i in run_first]
        wgc = np.stack([np.stack([Ws[l][cat] for l in range(L)])
                        for cat in cats])                     # [R, L, D, D]
        # bias packed [P, L*R*KT]: col (l*R+r)*KT+m, partition p ->
        # bs[l][cat_r][m*128+p]
        bgc = np.empty((P, L * R * KT), np.float32)
        for l in range(L):
            for r, cat in enumerate(cats):
                bgc[:, (l * R + r) * KT:(l * R + r + 1) * KT] = \
                    bs[l][cat].reshape(KT, P).T
        in_maps.append({
            "xT": xTc.astype(ACT_NP),
            "wg": np.ascontiguousarray(wgc).astype(ACT_NP),
            "bg": bgc,
        })
    return in_maps


def finish_output(results, order, B):
    out = np.empty((B, TOK, D), np.float32)
    for c in range(NCORES):
        outTc = np.asarray(results[c]["outT"], dtype=np.float32)
        out[order[c * S:(c + 1) * S]] = outTc.T.reshape(S, TOK, D)
    return out


def kernel(x, cat_ids, W1, b1, W2, b2, W3, b3, W4, b4):
    global LAST_RESULT
    cat_ids = np.asarray(cat_ids).astype(np.int64)
    Ws = [np.asarray(w, dtype=np.float32) for w in (W1, W2, W3, W4)]
    bs = [np.asarray(b, dtype=np.float32) for b in (b1, b2, b3, b4)]
    x = np.asarray(x, dtype=np.float32)
    B = x.shape[0]

    order, profile = plan(cat_ids)
    in_maps = prepare_in_maps(x, cat_ids, Ws, bs, order, profile)

    if profile not in _PROGRAM_CACHE:
        _PROGRAM_CACHE[profile] = build_program(profile=profile)
    nc = _PROGRAM_CACHE[profile]

    res = run_bass_kernel_spmd(nc, in_maps, list(range(NCORES)))
    LAST_RESULT = res
    return finish_output(res.results, order, B)
